# revision 3
# baseline (speedup 1.0000x reference)
"""FPN RoIAlign pooler (torchvision MultiScaleRoIAlign semantics) on 8 Trainium2
NeuronCores.

Strategy (data parallel over RoIs, per the sharding hint):
  - Host: route each of the 1024 RoIs to its FPN level, compute all bilinear
    sample coordinates and weights (exactly mirroring the reference math in
    fp32), and emit, per core, a stream of "gather slots". Each slot fetches a
    horizontally-adjacent pixel PAIR (channels contiguous in an NHWC copy of
    the features) and carries two weights (for the x-low / x-high taps, with
    the 2x2-sample averaging and empty-box masking folded in).
  - Device (per core, identical program): dma_gather streams slot rows into
    SBUF [128 slots, 2*256ch]; per 128-slot tile two sparse weight matrices
    (block pattern x per-slot scalar, built on DVE/ACT) reduce 8 slots -> 1
    output bin via TensorE matmul accumulation into PSUM [128 bins, 256 ch];
    PSUM tiles are copied to SBUF and DMA'd out as a [bins, 256] stream.
  - Host: un-permute bins back to [1024, 256, 7, 7].

dma_gather indices are int16, so each gather instruction reads from a <=32768
row window of the level table; level-0 (80000 pixel rows) is covered by two
overlapping windows per image (RoI y-span at level 0 is < 123 rows, so every
RoI fits one window).
"""

import os
import hashlib
import numpy as np

PH = PW = 7
SR = 2
SCALES = (0.25, 0.125, 0.0625, 0.03125)
LEVEL_HW = ((200, 200), (100, 100), (50, 50), (25, 25))
B = 2
N_CORES = 8
C = 256
SLOTS_PER_BIN = 8          # 2x2 samples x 2 row-taps (each fetches a pixel pair)
SLOTS_PER_ROI = 49 * SLOTS_PER_BIN
WIN = 32768                # int16 index window (rows)
F32 = np.float32

_DTYPE = os.environ.get("POOLER_DTYPE", "fp16")  # "fp16" | "fp32"


# ---------------------------------------------------------------- host: geometry
def _axis_terms(lo, span_px, size, scale):
    """Mirror of reference _roi_align/_bilinear for one axis, fp32.

    lo, span_px: [R] box low coord and size in image coords (pre-scale).
    Returns (low_idx, high_idx, w_low, w_high, empty) each [R, PH*SR]."""
    lo = (lo * F32(scale)).astype(F32)
    hi = (span_px * F32(scale)).astype(F32)  # this is hi coordinate, see caller
    roi = np.maximum(hi - lo, F32(1.0))
    bin_ = roi / F32(PH)
    i = np.arange(PH * SR)
    coord = (lo[:, None]
             + (i // SR)[None, :].astype(F32) * bin_[:, None]
             + ((i % SR) + 0.5)[None, :].astype(F32) * (bin_[:, None] / F32(SR)))
    empty = (coord < -1.0) | (coord > size)
    x = np.maximum(coord, F32(0.0))
    xl0 = np.floor(x)
    xc = xl0 >= size - 1
    x = np.where(xc, F32(size - 1), x)
    xl = np.where(xc, size - 1, xl0).astype(np.int32)
    xh = np.where(xc, size - 1, xl0 + 1).astype(np.int32)
    lx = x - xl.astype(F32)
    hx = F32(1.0) - lx
    return xl, xh, lx, hx, empty


def _route_levels(rois):
    area = (rois[:, 2] - rois[:, 0]) * (rois[:, 3] - rois[:, 1])
    s = np.sqrt(area.astype(F32))
    lvl = np.floor(F32(4.0) + np.log2(s / F32(224.0) + F32(1e-6)))
    return (np.clip(lvl, 2.0, 5.0).astype(np.int32) - 2)


def _build_slots(boxes):
    """Per-RoI slot indices / weights.

    Returns dict with, per RoI r: level[r], idx[r, 392] int32 (row index into
    the RoI's level table, pixel-pair base), wA/wB [r, 392] fp32."""
    boxes = np.asarray(boxes, F32)
    rois = boxes.reshape(B * 512, 4)
    bidx = np.repeat(np.arange(B, dtype=np.int32), 512)
    lvl = _route_levels(rois)
    R = rois.shape[0]
    idx = np.zeros((R, SLOTS_PER_ROI), np.int32)
    wA = np.zeros((R, SLOTS_PER_ROI), F32)
    wB = np.zeros((R, SLOTS_PER_ROI), F32)
    lo_idx = np.zeros(R, np.int64)
    hi_idx = np.zeros(R, np.int64)
    for l, ((H, W), scale) in enumerate(zip(LEVEL_HW, SCALES)):
        sel = np.nonzero(lvl == l)[0]
        if sel.size == 0:
            continue
        r = rois[sel]
        yl, yh, ly, hy, ey = _axis_terms(r[:, 1], r[:, 3], H, scale)
        xl, xh, lx, hx, ex = _axis_terms(r[:, 0], r[:, 2], W, scale)
        Rl = sel.size
        base = bidx[sel].astype(np.int64) * (H * W)
        y_samp = np.arange(PH * SR).reshape(PH, SR)      # iy = 2*py + sy
        x_samp = np.arange(PW * SR).reshape(PW, SR)      # ix = 2*px + sx
        # [Rl, py, px, sy, sx]
        row_l = yl[:, y_samp][:, :, None, :, None]
        row_h = yh[:, y_samp][:, :, None, :, None]
        w_l = hy[:, y_samp][:, :, None, :, None]
        w_h = ly[:, y_samp][:, :, None, :, None]
        col = xl[:, x_samp][:, None, :, None, :]
        w_hx = hx[:, x_samp][:, None, :, None, :]
        w_lx = lx[:, x_samp][:, None, :, None, :]
        emp = (ey[:, y_samp][:, :, None, :, None]
               | ex[:, x_samp][:, None, :, None, :])
        shape5 = (Rl, PH, PW, SR, SR)
        row_l = np.broadcast_to(row_l, shape5)
        row_h = np.broadcast_to(row_h, shape5)
        w_l = np.broadcast_to(w_l, shape5).astype(F32)
        w_h = np.broadcast_to(w_h, shape5).astype(F32)
        col = np.broadcast_to(col, shape5)
        w_hx = np.broadcast_to(w_hx, shape5).astype(F32)
        w_lx = np.broadcast_to(w_lx, shape5).astype(F32)
        valid = (~np.broadcast_to(emp, shape5)).astype(F32)
        # slots [Rl, 7, 7, 2, 2, 2]: t=0 -> row_l, t=1 -> row_h
        rows_t = np.stack([row_l, row_h], -1)
        wrow_t = np.stack([w_l, w_h], -1)
        idx6 = (base[:, None, None, None, None, None]
                + rows_t.astype(np.int64) * W
                + col.astype(np.int64)[..., None])
        q = F32(0.25) * wrow_t * valid[..., None]
        wA6 = q * w_hx[..., None]
        wB6 = q * w_lx[..., None]
        idx[sel] = idx6.reshape(Rl, -1).astype(np.int32)
        wA[sel] = wA6.reshape(Rl, -1)
        wB[sel] = wB6.reshape(Rl, -1)
        lo_idx[sel] = idx6.reshape(Rl, -1).min(1)
        hi_idx[sel] = idx6.reshape(Rl, -1).max(1)
    return dict(lvl=lvl, idx=idx, wA=wA, wB=wB, lo=lo_idx, hi=hi_idx)


def _segment_of(lvl, lo, hi):
    """Segment key per RoI: (level, window_base). Levels 1-3: base 0. Level 0:
    two overlapping windows per image."""
    R = lvl.shape[0]
    base = np.zeros(R, np.int64)
    l0 = lvl == 0
    H, W = LEVEL_HW[0]
    img = lo[l0] // (H * W)          # image id (RoI never crosses images)
    img_base = img * (H * W)
    rel_hi = hi[l0] - img_base
    w1 = H * W - WIN                 # second window start (7232)
    use_w1 = rel_hi > WIN - 1
    b = img_base + np.where(use_w1, w1, 0)
    assert (lo[l0] - b >= 0).all() and (hi[l0] - b <= WIN - 1).all(), "window overflow"
    base[l0] = b
    return base


# ---------------------------------------------------------------- host: streams
def _plan(boxes):
    """Compute the full per-core launch plan. Returns (meta, per_core_arrays).

    meta is hashable structure info (identical across cores) used to build the
    Bass program; arrays are the per-core input tensors."""
    sl = _build_slots(boxes)
    lvl, base = sl["lvl"], _segment_of(sl["lvl"], sl["lo"], sl["hi"])
    seg_keys = sorted(set(zip(lvl.tolist(), base.tolist())))
    seg_id = {k: i for i, k in enumerate(seg_keys)}
    rseg = np.array([seg_id[(int(l), int(b))] for l, b in zip(lvl, base)])
    order = np.lexsort((np.arange(lvl.size), rseg))
    cores = [order[c::N_CORES] for c in range(N_CORES)]

    # per (core, segment) RoI counts -> common capacities
    nseg = len(seg_keys)
    counts = np.zeros((N_CORES, nseg), np.int64)
    for c in range(N_CORES):
        for r in cores[c]:
            counts[c, rseg[r]] += 1
    cap_rois = counts.max(0)
    cap_slots = ((cap_rois * SLOTS_PER_ROI + 127) // 128) * 128
    total = int(cap_slots.sum())
    pad_tail = (-total) % 1024
    cap_slots[-1] += pad_tail
    total += pad_tail
    seg_off = np.concatenate([[0], np.cumsum(cap_slots)])

    # chunk list: per segment, gather instructions of <=1024 slots
    chunks = []  # (seg, n_slots, slot_off)
    for s in range(nseg):
        off = int(seg_off[s])
        left = int(cap_slots[s])
        while left > 0:
            n = min(1024, left)
            chunks.append((s, n, off))
            off += n
            left -= n

    # windows: rows available per segment (for in_ap row count + idx asserts)
    seg_rows = []
    for (l, b) in seg_keys:
        H, W = LEVEL_HW[l]
        seg_rows.append(int(min(WIN, B * H * W + 1 - b)))

    meta = dict(
        seg_keys=[(int(l), int(b)) for (l, b) in seg_keys],
        seg_rows=seg_rows,
        cap_slots=[int(x) for x in cap_slots],
        chunks=chunks,
        total=total,
        dtype=_DTYPE,
    )

    # per-core streams
    per_core = []
    for c in range(N_CORES):
        idx_s = np.zeros(total, np.int32)
        wA_s = np.zeros(total, F32)
        wB_s = np.zeros(total, F32)
        bin_pos = {}
        fill = seg_off[:-1].copy()
        for r in cores[c]:
            s = rseg[r]
            o = int(fill[s])
            idx_s[o:o + SLOTS_PER_ROI] = sl["idx"][r] - seg_keys[s][1]
            wA_s[o:o + SLOTS_PER_ROI] = sl["wA"][r]
            wB_s[o:o + SLOTS_PER_ROI] = sl["wB"][r]
            bin_pos[int(r)] = o // SLOTS_PER_BIN
            fill[s] += SLOTS_PER_ROI
        per_core.append(dict(idx=idx_s, wA=wA_s, wB=wB_s,
                             rois=cores[c], bin_pos=bin_pos))
    return meta, per_core


# ---------------------------------------------------------------- device program
_PROG_CACHE = {}


def _build_program(meta):
    import concourse.bass as bass
    import concourse.bacc as bacc
    import concourse.mybir as mybir
    from concourse import tile, library_config

    f32 = mybir.dt.float32
    i16 = mybir.dt.int16
    ft = mybir.dt.float16 if meta["dtype"] == "fp16" else f32
    esz = 2 if meta["dtype"] == "fp16" else 4

    total = meta["total"]
    NT = total // 128
    NBINS = total // SLOTS_PER_BIN
    chunks = meta["chunks"]
    seg_keys = meta["seg_keys"]
    seg_rows = meta["seg_rows"]

    nc = bacc.Bacc("TRN2", debug=False)
    tbls = []
    for l, (H, W) in enumerate(LEVEL_HW):
        rows = B * H * W + 16
        tbls.append(nc.dram_tensor(f"tbl{l}", [rows * C], ft, kind="ExternalInput"))
    idx_d = nc.dram_tensor("idx", [128, total // 16], i16, kind="ExternalInput")
    wts_d = nc.dram_tensor("wts", [128, 2 * NT], f32, kind="ExternalInput")
    msk_d = nc.dram_tensor("msk", [128, 8 * 128], ft, kind="ExternalInput")
    out_d = nc.dram_tensor("out", [NBINS, C], f32, kind="ExternalOutput")

    with tile.TileContext(nc) as tc:
        nc.gpsimd.load_library(library_config.mlp)
        with (
            tc.tile_pool(name="const", bufs=1) as cpool,
            tc.tile_pool(name="g", bufs=4) as gpool,
            tc.tile_pool(name="m", bufs=8) as mpool,
            tc.tile_pool(name="st", bufs=3) as spool,
            tc.tile_pool(name="ps", bufs=4, space="PSUM") as ppool,
        ):
            idx_t = cpool.tile([128, total // 16], i16)
            nc.sync.dma_start(out=idx_t[:], in_=idx_d[:])
            wts_t = cpool.tile([128, 2 * NT], f32)
            nc.sync.dma_start(out=wts_t[:], in_=wts_d[:])
            msk_t = cpool.tile([128, 8 * 128], ft)
            nc.sync.dma_start(out=msk_t[:], in_=msk_d[:])

            wins = [
                bass.AP(tbls[l][:].tensor, b * C, [[C, seg_rows[s]], [1, 2 * C]])
                for s, (l, b) in enumerate(seg_keys)
            ]

            psum = None
            for (s, n, off) in chunks:
                k = n // 128
                g_t = gpool.tile([128, 8 * 2 * C], ft, tag="g")
                g3 = g_t[:, : k * 2 * C].rearrange("p (k e) -> p k e", e=2 * C)
                nc.gpsimd.dma_gather(
                    g3, wins[s], idx_t[:, off // 16: (off + n) // 16],
                    n, n, 2 * C, elem_step=C,
                )
                for j in range(k):
                    g = off // 128 + j
                    jj = g % 8
                    if jj == 0:
                        psum = ppool.tile([128, C], f32, space="PSUM", tag="ps")
                    mA = mpool.tile([128, 128], ft, tag="m")
                    mB = mpool.tile([128, 128], ft, tag="m")
                    nc.vector.tensor_scalar_mul(
                        mA[:], msk_t[:, jj * 128:(jj + 1) * 128],
                        wts_t[:, 2 * g:2 * g + 1])
                    nc.scalar.mul(
                        mB[:], msk_t[:, jj * 128:(jj + 1) * 128],
                        wts_t[:, 2 * g + 1:2 * g + 2])
                    nc.tensor.matmul(psum[:], lhsT=mA[:], rhs=g3[:, j, 0:C],
                                     start=(jj == 0), stop=False)
                    nc.tensor.matmul(psum[:], lhsT=mB[:], rhs=g3[:, j, C:2 * C],
                                     start=False, stop=(jj == 7))
                    if jj == 7:
                        pt = g // 8
                        st = spool.tile([128, C], f32, tag="st")
                        nc.vector.tensor_copy(st[:], psum[:])
                        nc.sync.dma_start(
                            out=out_d[pt * 128:(pt + 1) * 128, :], in_=st[:])
    nc.compile()
    return nc


# ---------------------------------------------------------------- entry point
def kernel(x0, x1, x2, x3, boxes):
    from concourse.bass_utils import run_bass_kernel_spmd

    feats = [np.asarray(a) for a in (x0, x1, x2, x3)]
    boxes = np.asarray(boxes, F32)
    out_dtype = feats[0].dtype

    meta, per_core = _plan(boxes)

    # NHWC tables (+pad rows), cast to compute dtype
    np_ft = np.float16 if meta["dtype"] == "fp16" else np.float32
    tbl_arrs = []
    for l, (H, W) in enumerate(LEVEL_HW):
        t = np.ascontiguousarray(
            feats[l].astype(np_ft).transpose(0, 2, 3, 1)).reshape(B * H * W, C)
        full = np.zeros((B * H * W + 16, C), np_ft)
        full[: B * H * W] = t
        tbl_arrs.append(full.reshape(-1))

    total = meta["total"]
    NT = total // 128
    masks = np.zeros((128, 8 * 128), np_ft)
    for j in range(8):
        for p in range(128):
            masks[p, j * 128 + 16 * j + p // 8] = 1.0

    in_maps = []
    for c in range(N_CORES):
        pc = per_core[c]
        idx16 = pc["idx"].astype(np.int16)
        assert (pc["idx"] >= 0).all() and (pc["idx"] < 32768).all()
        idx_w = np.tile(idx16.reshape(-1, 16).T, (8, 1))
        wts = np.zeros((128, 2 * NT), F32)
        wts[:, 0::2] = pc["wA"].reshape(NT, 128).T
        wts[:, 1::2] = pc["wB"].reshape(NT, 128).T
        m = {f"tbl{l}": tbl_arrs[l] for l in range(4)}
        m.update(idx=np.ascontiguousarray(idx_w),
                 wts=np.ascontiguousarray(wts), msk=masks)
        in_maps.append(m)

    key = hashlib.sha256(repr(sorted(meta.items())).encode()).hexdigest()
    if key not in _PROG_CACHE:
        _PROG_CACHE[key] = _build_program(meta)
    nc = _PROG_CACHE[key]

    res = run_bass_kernel_spmd(nc, in_maps, core_ids=list(range(N_CORES)),
                               trace=bool(int(os.environ.get("POOLER_TRACE", "0"))))
    kernel.last_result = res

    final = np.empty((B * 512, 49, C), F32)
    for c in range(N_CORES):
        out = res.results[c]["out"]
        pc = per_core[c]
        rois = pc["rois"]
        pos = np.array([pc["bin_pos"][int(r)] for r in rois])[:, None] + np.arange(49)[None, :]
        final[rois] = out[pos]
    return final.transpose(0, 2, 1).reshape(B * 512, C, PH, PW).astype(out_dtype)
